# revision 7
# baseline (speedup 1.0000x reference)
# Trainium2 Bass kernel for: ConvTranspose2d(64->128, k=4, stride=1) -> spatial
# mean -> +biases -> 10*logsumexp over channels.
#
# Math: with full (K-1) output padding, the mean over the ENTIRE conv-transpose
# output spatial extent sees every input pixel through all K*K taps, so
#   pooled[n,co] = (sum_hw x[n,ci,hw]) @ (sum_kk w[ci,co,kk]) / (Ho*Wo) + cb + eb
# exactly. The conv collapses to a spatial sum + a (Cin x Cout) matmul.
#
# Sharding: data-parallel over batch N=32 across 8 cores (4 batches/core).
#
# Trace-driven design (see transcript):
# - x quantized to fp8 e4m3 on host (1 MiB/core, 4x less HBM traffic; final
#   output error ~1e-4 rel vs the 2e-2 gate since fp8 noise averages over the
#   4096-wide spatial sum).
# - Host pre-transposes x so (ci, hw%2) sits on partitions; the spatial sum
#   runs on the PE as a block-mask matmul. DoubleRow perf mode contracts two
#   256-column k-tiles per instruction (2 cols/cycle), so the PE tracks the
#   DMA stream even in the half-rate window the trace shows while SDMA writes
#   are in flight.
# - x rides BOTH HWDGE rings (2 chunks on SP, 2 on ACT) to reach the ~345 GB/s
#   HBM ceiling sooner; params go first on the ACT ring (they gated the first
#   matmul by 1.2us when queued behind the 1.3us ACT_TABLE_LOAD).
# - bias row is folded into the stage-2 matmul as a 65th contraction row of
#   wsum, removing a separate fp32 matmul (fp32 lowers to a slow LOW/HIGH
#   double pass on the PE).
# - One pre-placed LoadActFuncSet covering BOTH Exp and Ln (emitted after the
#   DMA issue instructions) keeps the 1.3us table load off the critical path.

import os

import ml_dtypes
import numpy as np

import concourse.bacc as bacc
import concourse.bass as bass
import concourse.mybir as mybir
import concourse.tile as tile
from concourse.bass_utils import run_bass_kernel_spmd
from concourse.hw_specs import get_activation_tables

N, CIN, COUT, K, H, W = 32, 64, 128, 4, 64, 64
NCORES = 8
NLOC = N // NCORES          # 4 batches per core
HW = H * W                  # 4096
SCALE = 1.0 / float((H + K - 1) * (W + K - 1))   # 1/4489

# x layout per core: xq[p, j], p = ci*2 + hw_lo, j = co_*256 + n*64 + ci_
# with hw = (co_*64 + ci_)*2 + hw_lo;  co_ = c_outer in [0,32), ci_ = c_inner.
COUT_CHUNKS = 32            # k-tiles accumulated in PSUM (c_outer)
CINNER = 64                 # folded by the DVE tail reduce
FD = NLOC * CINNER          # 256 columns per k-tile
XCOLS = COUT_CHUNKS * FD    # 8192
MCOLS = 2 * CIN             # mask columns embedded at the head of chunk 0
NDMA = 4                    # x DMA chunks
DMACOLS = XCOLS // NDMA     # 2048
MMPD = COUT_CHUNKS // NDMA // 2   # DoubleRow matmuls per DMA chunk (4)

F32 = mybir.dt.float32
BF16 = mybir.dt.bfloat16
F8 = mybir.dt.float8e4
NP_F8 = ml_dtypes.float8_e4m3
NP_BF16 = ml_dtypes.bfloat16

_CACHE: dict = {}


def _build_module() -> bacc.Bacc:
    nc = bacc.Bacc("TRN2", target_bir_lowering=False, enable_partition_id=False)

    x_d = nc.dram_tensor("xq", [128, MCOLS + XCOLS], F8, kind="ExternalInput").ap()
    w_d = nc.dram_tensor("wse", [CIN + 1, COUT], BF16, kind="ExternalInput").ap()
    y_d = nc.dram_tensor("y", [NLOC, 1], F32, kind="ExternalOutput").ap()

    with tile.TileContext(nc) as tc:
        with (
            tc.tile_pool(name="xpool", bufs=NDMA) as xpool,
            tc.tile_pool(name="small", bufs=1) as small,
            tc.tile_pool(name="ps1", bufs=1, space="PSUM") as ps1,
            tc.tile_pool(name="ps2", bufs=1, space="PSUM") as ps2,
        ):
            # stage-2 lhsT: rows 0..63 get the spatial sums, row 64 is the
            # all-ones row that pulls in the bias row of wse.
            sT = small.tile([CIN + 1, NLOC], BF16)
            nc.vector.memset(sT[CIN : CIN + 1, :], 1.0)

            # ---- stage 1: spatial sums on the PE (fp8 DoubleRow) ----
            # The mask rides at the head of chunk 0 (a separate tiny-
            # descriptor param DMA completed 2us late and gated matmul 0).
            # Chunks alternate SP/ACT HWDGE rings so both pull concurrently;
            # PE program order matches arrival order.
            # P[ci, n*64 + ci_] accumulates sum over (hw_lo, c_outer).
            P = ps1.tile([CIN, FD], F32, space="PSUM")
            mask3 = None
            for k in range(NDMA):
                cols = (MCOLS if k == 0 else 0) + DMACOLS
                off = 0 if k == 0 else MCOLS + k * DMACOLS
                xt = xpool.tile([128, cols], F8)
                eng = nc.sync if k % 2 == 0 else nc.scalar
                eng.dma_start(out=xt, in_=x_d[:, off : off + cols])
                if k == 0:
                    mask3 = xt[:, 0:MCOLS].rearrange("p (k i) -> p k i", k=2)
                    xoff = MCOLS
                else:
                    xoff = 0
                for c in range(MMPD):
                    rhs3 = xt[
                        :, xoff + 2 * c * FD : xoff + 2 * (c + 1) * FD
                    ].rearrange("p (kk j) -> p kk j", kk=2)
                    nc.tensor.matmul(
                        out=P,
                        lhsT=mask3,
                        rhs=rhs3,
                        start=(k == 0 and c == 0),
                        stop=(k == NDMA - 1 and c == MMPD - 1),
                        perf_mode=mybir.MatmulPerfMode.DoubleRow,
                    )

            # wse on the sync ring after its x chunks (its 65-partition
            # descriptor gen is slow and must not delay the x stream).
            wset = small.tile([CIN + 1, COUT], BF16)
            nc.sync.dma_start(out=wset, in_=w_d)

            # NOTE: no manual InstLoadActFuncSet — the insert_act_table_loads
            # pass places one Exp+Ln load after the scalar ring's DMA issue
            # instructions, off the critical path (trace-verified).

            # ---- fold c_inner: sT[ci, n] = sum_ci_ P[ci, n*64+ci_] ----
            with nc.allow_low_precision(
                reason="S feeds a 64-deep bf16 matmul; fp8 input noise dominates"
            ):
                nc.vector.reduce_sum(
                    out=sT[0:CIN, :],
                    in_=P.rearrange("p (n c) -> p n c", n=NLOC),
                    axis=mybir.AxisListType.X,
                )

            # ---- stage 2: pooled[n, co] = sT.T @ wse (bias folded) ----
            pooled = ps2.tile([NLOC, COUT], F32, space="PSUM")
            nc.tensor.matmul(out=pooled, lhsT=sT, rhs=wset, start=True, stop=True)

            # ---- 10 * log(sum_co exp(pooled)) on ACT ----
            expt = small.tile([NLOC, COUT], F32)
            sume = small.tile([NLOC, 1], F32)
            nc.scalar.activation(
                out=expt,
                in_=pooled,
                func=mybir.ActivationFunctionType.Exp,
                accum_out=sume,
            )
            logv = small.tile([NLOC, 1], F32)
            nc.scalar.activation(
                out=logv, in_=sume, func=mybir.ActivationFunctionType.Ln
            )
            outv = small.tile([NLOC, 1], F32)
            nc.scalar.mul(out=outv, in_=logv, mul=10.0)
            nc.sync.dma_start(out=y_d, in_=outv)

    nc.compile()
    return nc


def _prep_inputs(x, weight, conv_bias, extra_bias):
    wse = np.empty((CIN + 1, COUT), dtype=np.float32)
    wse[:CIN] = weight.sum(axis=(2, 3)) * SCALE
    wse[CIN] = conv_bias + extra_bias
    wse = wse.astype(NP_BF16)
    # mask[p, k*64 + i] = (p//2 == i), duplicated over the two k-tiles
    mask = np.zeros((128, MCOLS), dtype=NP_F8)
    for kk in range(2):
        mask[np.arange(128), kk * CIN + np.arange(128) // 2] = 1.0
    in_maps = []
    for c in range(NCORES):
        xs = x[c * NLOC : (c + 1) * NLOC]                          # (4,64,64,64)
        # (n, ci, co_, ci_, hw_lo) -> (ci, hw_lo, co_, n, ci_)
        x5 = xs.reshape(NLOC, CIN, COUT_CHUNKS, CINNER, 2)
        xq = np.empty((128, MCOLS + XCOLS), dtype=NP_F8)
        xq[:, :MCOLS] = mask
        xq[:, MCOLS:] = x5.transpose(1, 4, 2, 0, 3).reshape(128, XCOLS)
        in_maps.append({"xq": xq, "wse": wse})
    return in_maps


def kernel(x, weight, conv_bias, extra_bias):
    x = np.ascontiguousarray(np.asarray(x, dtype=np.float32))
    weight = np.ascontiguousarray(np.asarray(weight, dtype=np.float32))
    conv_bias = np.asarray(conv_bias, dtype=np.float32)
    extra_bias = np.asarray(extra_bias, dtype=np.float32)
    assert x.shape == (N, CIN, H, W), x.shape
    assert weight.shape == (CIN, COUT, K, K), weight.shape

    if "nc" not in _CACHE:
        _CACHE["nc"] = _build_module()
    nc = _CACHE["nc"]

    in_maps = _prep_inputs(x, weight, conv_bias, extra_bias)

    trace = os.environ.get("BASS_KERNEL_TRACE") == "1"
    res = run_bass_kernel_spmd(
        nc, in_maps, core_ids=list(range(NCORES)), trace=trace
    )
    _CACHE["last_result"] = res
    return np.concatenate([r["y"] for r in res.results], axis=0)


# revision 10
# speedup vs baseline: 1.0578x; 1.0578x over previous
# Trainium2 Bass kernel for: ConvTranspose2d(64->128, k=4, stride=1) -> spatial
# mean -> +biases -> 10*logsumexp over channels.
#
# Math: with full (K-1) output padding, the mean over the ENTIRE conv-transpose
# output spatial extent sees every input pixel through all K*K taps, so
#   pooled[n,co] = (sum_hw x[n,ci,hw]) @ (sum_kk w[ci,co,kk]) / (Ho*Wo) + cb + eb
# exactly. The conv collapses to a spatial sum + a (Cin x Cout) matmul.
#
# Sharding: data-parallel over batch N=32 across 8 cores (4 batches/core).
#
# Trace-driven design (see transcript):
# - x quantized to fp8 e4m3 on host (1 MiB/core, 4x less HBM traffic; final
#   output error ~1e-4 rel vs the 2e-2 gate since fp8 noise averages over the
#   4096-wide spatial sum).
# - Host pre-transposes x so (ci, hw%2) sits on partitions; the spatial sum
#   runs on the PE as a block-mask matmul. DoubleRow perf mode contracts two
#   256-column k-tiles per instruction (2 cols/cycle), so the PE tracks the
#   DMA stream even in the half-rate window the trace shows while SDMA writes
#   are in flight.
# - x rides BOTH HWDGE rings (2 chunks on SP, 2 on ACT) to reach the ~345 GB/s
#   HBM ceiling sooner; params go first on the ACT ring (they gated the first
#   matmul by 1.2us when queued behind the 1.3us ACT_TABLE_LOAD).
# - bias row is folded into the stage-2 matmul as a 65th contraction row of
#   wsum, removing a separate fp32 matmul (fp32 lowers to a slow LOW/HIGH
#   double pass on the PE).
# - One pre-placed LoadActFuncSet covering BOTH Exp and Ln (emitted after the
#   DMA issue instructions) keeps the 1.3us table load off the critical path.

import os

import ml_dtypes
import numpy as np

import concourse.bacc as bacc
import concourse.bass as bass
import concourse.mybir as mybir
import concourse.tile as tile
from concourse.bass_utils import run_bass_kernel_spmd
from concourse.hw_specs import get_activation_tables

N, CIN, COUT, K, H, W = 32, 64, 128, 4, 64, 64
NCORES = 8
NLOC = N // NCORES          # 4 batches per core
HW = H * W                  # 4096
SCALE = 1.0 / float((H + K - 1) * (W + K - 1))   # 1/4489

# x layout per core: xq[p, j], p = ci*2 + hw_lo, j = co_*256 + n*64 + ci_
# with hw = (co_*64 + ci_)*2 + hw_lo;  co_ = c_outer in [0,32), ci_ = c_inner.
COUT_CHUNKS = 32            # k-tiles accumulated in PSUM (c_outer)
CINNER = 64                 # folded by the DVE tail reduce
FD = NLOC * CINNER          # 256 columns per k-tile
XCOLS = COUT_CHUNKS * FD    # 8192
MCOLS = 2 * CIN             # mask columns embedded at the head of chunk 0
# x chunk sizes in DoubleRow-matmul units (512 cols each). Chunk 0 is small so
# the PE starts early; all x chunks ride ONE HWDGE ring (FIFO) so completions
# are ordered -- concurrent queues round-robin on the shared SDMA engines and
# delay the first completion.
MMS = [1, 4, 4, 4, 3]       # sums to 16 (= 8192 cols / 512)

F32 = mybir.dt.float32
BF16 = mybir.dt.bfloat16
F8 = mybir.dt.float8e4
NP_F8 = ml_dtypes.float8_e4m3
NP_BF16 = ml_dtypes.bfloat16

_CACHE: dict = {}


def _build_module() -> bacc.Bacc:
    nc = bacc.Bacc("TRN2", target_bir_lowering=False, enable_partition_id=False)

    x_d = nc.dram_tensor("xq", [128, MCOLS + XCOLS], F8, kind="ExternalInput").ap()
    w_d = nc.dram_tensor("wse", [CIN + 1, COUT], BF16, kind="ExternalInput").ap()
    y_d = nc.dram_tensor("y", [NLOC, 1], F32, kind="ExternalOutput").ap()

    with tile.TileContext(nc) as tc:
        with (
            tc.tile_pool(name="xpool", bufs=len(MMS)) as xpool,
            tc.tile_pool(name="small", bufs=1) as small,
            tc.tile_pool(name="ps1", bufs=1, space="PSUM") as ps1,
            tc.tile_pool(name="ps2", bufs=1, space="PSUM") as ps2,
        ):
            # One ACT table set covering BOTH Exp and Ln, pre-placed so the
            # insert_act_table_loads pass doesn't split them into two sets
            # and drop a 1.3us load between EXP and LN on the critical tail
            # (trace-verified in a run without this). The load is
            # non-blocking at program start.
            act_tables = get_activation_tables(nc.m.arch)
            set_id = next(
                i
                for i, (_, funcs) in enumerate(act_tables.items())
                if mybir.ActivationFunctionType.Exp in funcs
                and mybir.ActivationFunctionType.Ln in funcs
            )
            nc.scalar.add_instruction(
                mybir.InstLoadActFuncSet(
                    name=nc.get_next_instruction_name(), act_func_set_id=set_id
                )
            )

            # wse on the ACT ring (its 65-partition descriptor gen is slow
            # and must not sit in front of the x chunks).
            wset = small.tile([CIN + 1, COUT], BF16)
            nc.scalar.dma_start(out=wset, in_=w_d)

            # stage-2 lhsT: rows 0..63 get the spatial sums, row 64 is the
            # all-ones row that pulls in the bias row of wse.
            sT = small.tile([CIN + 1, NLOC], BF16)
            nc.vector.memset(sT[CIN : CIN + 1, :], 1.0)

            # ---- stage 1: spatial sums on the PE (fp8 DoubleRow) ----
            # The mask rides at the head of chunk 0 (a separate tiny-
            # descriptor param DMA completed 2us late and gated matmul 0).
            # P[ci, n*64 + ci_] accumulates sum over (hw_lo, c_outer).
            P = ps1.tile([CIN, FD], F32, space="PSUM")
            mask3 = None
            off = 0
            done = 0
            for k, nmm in enumerate(MMS):
                cols = (MCOLS if k == 0 else 0) + nmm * 2 * FD
                xt = xpool.tile([128, cols], F8)
                nc.sync.dma_start(out=xt, in_=x_d[:, off : off + cols])
                off += cols
                xoff = MCOLS if k == 0 else 0
                if k == 0:
                    mask3 = xt[:, 0:MCOLS].rearrange("p (k i) -> p k i", k=2)
                for c in range(nmm):
                    rhs3 = xt[
                        :, xoff + 2 * c * FD : xoff + 2 * (c + 1) * FD
                    ].rearrange("p (kk j) -> p kk j", kk=2)
                    nc.tensor.matmul(
                        out=P,
                        lhsT=mask3,
                        rhs=rhs3,
                        start=(done == 0),
                        stop=(done == COUT_CHUNKS // 2 - 1),
                        perf_mode=mybir.MatmulPerfMode.DoubleRow,
                    )
                    done += 1

            # ---- fold c_inner: sT[ci, n] = sum_ci_ P[ci, n*64+ci_] ----
            with nc.allow_low_precision(
                reason="S feeds a 64-deep bf16 matmul; fp8 input noise dominates"
            ):
                nc.vector.reduce_sum(
                    out=sT[0:CIN, :],
                    in_=P.rearrange("p (n c) -> p n c", n=NLOC),
                    axis=mybir.AxisListType.X,
                )

            # ---- stage 2: pooled[n, co] = sT.T @ wse (bias folded) ----
            pooled = ps2.tile([NLOC, COUT], F32, space="PSUM")
            nc.tensor.matmul(out=pooled, lhsT=sT, rhs=wset, start=True, stop=True)

            # ---- 10 * log(sum_co exp(pooled)) on ACT ----
            expt = small.tile([NLOC, COUT], F32)
            sume = small.tile([NLOC, 1], F32)
            nc.scalar.activation(
                out=expt,
                in_=pooled,
                func=mybir.ActivationFunctionType.Exp,
                accum_out=sume,
            )
            logv = small.tile([NLOC, 1], F32)
            nc.scalar.activation(
                out=logv, in_=sume, func=mybir.ActivationFunctionType.Ln
            )
            outv = small.tile([NLOC, 1], F32)
            nc.scalar.mul(out=outv, in_=logv, mul=10.0)
            # y rides the ACT ring: same engine as the preceding ops, so no
            # cross-engine semaphore hop before the final descriptor gen.
            nc.scalar.dma_start(out=y_d, in_=outv)

    nc.compile()
    return nc


def _prep_inputs(x, weight, conv_bias, extra_bias):
    wse = np.empty((CIN + 1, COUT), dtype=np.float32)
    wse[:CIN] = weight.sum(axis=(2, 3)) * SCALE
    wse[CIN] = conv_bias + extra_bias
    wse = wse.astype(NP_BF16)
    # mask[p, k*64 + i] = (p//2 == i), duplicated over the two k-tiles
    mask = np.zeros((128, MCOLS), dtype=NP_F8)
    for kk in range(2):
        mask[np.arange(128), kk * CIN + np.arange(128) // 2] = 1.0
    in_maps = []
    for c in range(NCORES):
        xs = x[c * NLOC : (c + 1) * NLOC]                          # (4,64,64,64)
        # (n, ci, co_, ci_, hw_lo) -> (ci, hw_lo, co_, n, ci_)
        x5 = xs.reshape(NLOC, CIN, COUT_CHUNKS, CINNER, 2)
        xq = np.empty((128, MCOLS + XCOLS), dtype=NP_F8)
        xq[:, :MCOLS] = mask
        xq[:, MCOLS:] = x5.transpose(1, 4, 2, 0, 3).reshape(128, XCOLS)
        in_maps.append({"xq": xq, "wse": wse})
    return in_maps


def kernel(x, weight, conv_bias, extra_bias):
    x = np.ascontiguousarray(np.asarray(x, dtype=np.float32))
    weight = np.ascontiguousarray(np.asarray(weight, dtype=np.float32))
    conv_bias = np.asarray(conv_bias, dtype=np.float32)
    extra_bias = np.asarray(extra_bias, dtype=np.float32)
    assert x.shape == (N, CIN, H, W), x.shape
    assert weight.shape == (CIN, COUT, K, K), weight.shape

    if "nc" not in _CACHE:
        _CACHE["nc"] = _build_module()
    nc = _CACHE["nc"]

    in_maps = _prep_inputs(x, weight, conv_bias, extra_bias)

    trace = os.environ.get("BASS_KERNEL_TRACE") == "1"
    res = run_bass_kernel_spmd(
        nc, in_maps, core_ids=list(range(NCORES)), trace=trace
    )
    _CACHE["last_result"] = res
    return np.concatenate([r["y"] for r in res.results], axis=0)


# revision 15
# speedup vs baseline: 1.0790x; 1.0200x over previous
# Trainium2 Bass kernel for: ConvTranspose2d(64->128, k=4, stride=1) -> spatial
# mean -> +biases -> 10*logsumexp over channels.
#
# Math: with full (K-1) output padding, the mean over the ENTIRE conv-transpose
# output spatial extent sees every input pixel through all K*K taps, so
#   pooled[n,co] = (sum_hw x[n,ci,hw]) @ (sum_kk w[ci,co,kk]) / (Ho*Wo) + cb + eb
# exactly. The conv collapses to a spatial sum + a (Cin x Cout) matmul.
#
# Sharding: data-parallel over batch N=32 across 8 cores (4 batches/core).
#
# Trace-driven design (see transcript):
# - x quantized to fp8 e4m3 on host (1 MiB/core, 4x less HBM traffic; final
#   output error ~1e-4 rel vs the 2e-2 gate since fp8 noise averages over the
#   4096-wide spatial sum).
# - Host pre-transposes x so (ci, hw%2) sits on partitions; the spatial sum
#   runs on the PE as a block-mask matmul. DoubleRow perf mode contracts two
#   256-column k-tiles per instruction (2 cols/cycle), so the PE tracks the
#   DMA stream even in the half-rate window the trace shows while SDMA writes
#   are in flight.
# - x rides BOTH HWDGE rings (2 chunks on SP, 2 on ACT) to reach the ~345 GB/s
#   HBM ceiling sooner; params go first on the ACT ring (they gated the first
#   matmul by 1.2us when queued behind the 1.3us ACT_TABLE_LOAD).
# - bias row is folded into the stage-2 matmul as a 65th contraction row of
#   wsum, removing a separate fp32 matmul (fp32 lowers to a slow LOW/HIGH
#   double pass on the PE).
# - One pre-placed LoadActFuncSet covering BOTH Exp and Ln (emitted after the
#   DMA issue instructions) keeps the 1.3us table load off the critical path.

import os

import ml_dtypes
import numpy as np

import concourse.bacc as bacc
import concourse.bass as bass
import concourse.mybir as mybir
import concourse.tile as tile
from concourse.bass_utils import run_bass_kernel_spmd
from concourse.hw_specs import get_activation_tables

N, CIN, COUT, K, H, W = 32, 64, 128, 4, 64, 64
NCORES = 8
NLOC = N // NCORES          # 4 batches per core
HW = H * W                  # 4096
SCALE = 1.0 / float((H + K - 1) * (W + K - 1))   # 1/4489

# x layout per core: xq[p, j], p = ci*2 + hw_lo, j = co_*256 + n*64 + ci_
# with hw = (co_*64 + ci_)*2 + hw_lo;  co_ = c_outer in [0,32), ci_ = c_inner.
COUT_CHUNKS = 32            # k-tiles accumulated in PSUM (c_outer)
CINNER = 64                 # folded by the DVE tail reduce
FD = NLOC * CINNER          # 256 columns per k-tile
XCOLS = COUT_CHUNKS * FD    # 8192
MCOLS = 2 * CIN             # mask columns embedded at the head of chunk 0
# x chunk sizes in DoubleRow-matmul units (512 cols each). Chunk 0 is small so
# the PE starts early; all x chunks ride ONE HWDGE ring (FIFO) so completions
# are ordered -- concurrent queues round-robin on the shared SDMA engines and
# delay the first completion.
MMS = [1, 5, 5, 4, 1]       # sums to 16 (= 8192 cols / 512)

F32 = mybir.dt.float32
BF16 = mybir.dt.bfloat16
F8 = mybir.dt.float8e4
NP_F8 = ml_dtypes.float8_e4m3
NP_BF16 = ml_dtypes.bfloat16

_CACHE: dict = {}


def _build_module() -> bacc.Bacc:
    nc = bacc.Bacc("TRN2", target_bir_lowering=False, enable_partition_id=False)

    x_d = nc.dram_tensor("xq", [128, MCOLS + XCOLS], F8, kind="ExternalInput").ap()
    # wse padded to 128 partitions: a 65-partition DMA costs ~1.4us of
    # descriptor gen vs ~0.7us for a full-partition one.
    w_d = nc.dram_tensor("wse", [128, COUT], BF16, kind="ExternalInput").ap()
    y_d = nc.dram_tensor("y", [NLOC, 1], F32, kind="ExternalOutput").ap()

    with tile.TileContext(nc) as tc:
        with (
            tc.tile_pool(name="xpool", bufs=len(MMS)) as xpool,
            tc.tile_pool(name="small", bufs=1) as small,
            tc.tile_pool(name="ps1", bufs=1, space="PSUM") as ps1,
            tc.tile_pool(name="ps2", bufs=1, space="PSUM") as ps2,
        ):
            # One ACT table set covering BOTH Exp and Ln, pre-placed so the
            # insert_act_table_loads pass doesn't split them into two sets
            # and drop a 1.3us load between EXP and LN on the critical tail
            # (trace-verified in a run without this). The load is
            # non-blocking at program start.
            act_tables = get_activation_tables(nc.m.arch)
            set_id = next(
                i
                for i, (_, funcs) in enumerate(act_tables.items())
                if mybir.ActivationFunctionType.Exp in funcs
                and mybir.ActivationFunctionType.Ln in funcs
            )
            nc.scalar.add_instruction(
                mybir.InstLoadActFuncSet(
                    name=nc.get_next_instruction_name(), act_func_set_id=set_id
                )
            )

            # wse on the ACT ring (its 65-partition descriptor gen is slow
            # and must not sit in front of the x chunks).
            wset = small.tile([128, COUT], BF16)
            nc.scalar.dma_start(out=wset, in_=w_d)

            # stage-2 lhsT: rows 0..63 get the spatial sums, row 64 is the
            # all-ones row that pulls in the bias row of wse.
            sT = small.tile([CIN + 1, NLOC], BF16)
            nc.vector.memset(sT[CIN : CIN + 1, :], 1.0)

            # ---- stage 1: spatial sums on the PE (fp8 DoubleRow) ----
            # The mask rides at the head of chunk 0 (a separate tiny-
            # descriptor param DMA completed 2us late and gated matmul 0).
            # P[ci, n*64 + ci_] accumulates sum over (hw_lo, c_outer).
            P = ps1.tile([CIN, FD], F32, space="PSUM")
            mask3 = None
            off = 0
            done = 0
            for k, nmm in enumerate(MMS):
                cols = (MCOLS if k == 0 else 0) + nmm * 2 * FD
                xt = xpool.tile([128, cols], F8)
                nc.sync.dma_start(out=xt, in_=x_d[:, off : off + cols])
                off += cols
                xoff = MCOLS if k == 0 else 0
                if k == 0:
                    mask3 = xt[:, 0:MCOLS].rearrange("p (k i) -> p k i", k=2)
                for c in range(nmm):
                    rhs3 = xt[
                        :, xoff + 2 * c * FD : xoff + 2 * (c + 1) * FD
                    ].rearrange("p (kk j) -> p kk j", kk=2)
                    nc.tensor.matmul(
                        out=P,
                        lhsT=mask3,
                        rhs=rhs3,
                        start=(done == 0),
                        stop=(done == COUT_CHUNKS // 2 - 1),
                        perf_mode=mybir.MatmulPerfMode.DoubleRow,
                    )
                    done += 1

            # ---- fold c_inner: sT[ci, n] = sum_ci_ P[ci, n*64+ci_] ----
            with nc.allow_low_precision(
                reason="S feeds a 64-deep bf16 matmul; fp8 input noise dominates"
            ):
                nc.vector.reduce_sum(
                    out=sT[0:CIN, :],
                    in_=P.rearrange("p (n c) -> p n c", n=NLOC),
                    axis=mybir.AxisListType.X,
                )

            # ---- stage 2: pooled[n, co] = sT.T @ wse (bias folded) ----
            pooled = ps2.tile([NLOC, COUT], F32, space="PSUM")
            nc.tensor.matmul(
                out=pooled, lhsT=sT, rhs=wset[0 : CIN + 1, :], start=True, stop=True
            )

            # ---- 10 * log(sum_co exp(pooled)) on ACT ----
            expt = small.tile([NLOC, COUT], F32)
            sume = small.tile([NLOC, 1], F32)
            nc.scalar.activation(
                out=expt,
                in_=pooled,
                func=mybir.ActivationFunctionType.Exp,
                accum_out=sume,
            )
            logv = small.tile([NLOC, 1], F32)
            nc.scalar.activation(
                out=logv, in_=sume, func=mybir.ActivationFunctionType.Ln
            )
            outv = small.tile([NLOC, 1], F32)
            nc.scalar.mul(out=outv, in_=logv, mul=10.0)
            # y rides the ACT ring: same engine as the preceding ops, so no
            # cross-engine semaphore hop before the final descriptor gen.
            nc.scalar.dma_start(out=y_d, in_=outv)

    nc.compile()
    return nc


def _prep_inputs(x, weight, conv_bias, extra_bias):
    wse = np.zeros((128, COUT), dtype=np.float32)
    wse[:CIN] = weight.sum(axis=(2, 3)) * SCALE
    wse[CIN] = conv_bias + extra_bias
    wse = wse.astype(NP_BF16)
    # mask[p, k*64 + i] = (p//2 == i), duplicated over the two k-tiles
    mask = np.zeros((128, MCOLS), dtype=NP_F8)
    for kk in range(2):
        mask[np.arange(128), kk * CIN + np.arange(128) // 2] = 1.0
    in_maps = []
    for c in range(NCORES):
        xs = x[c * NLOC : (c + 1) * NLOC]                          # (4,64,64,64)
        # (n, ci, co_, ci_, hw_lo) -> (ci, hw_lo, co_, n, ci_)
        x5 = xs.reshape(NLOC, CIN, COUT_CHUNKS, CINNER, 2)
        xq = np.empty((128, MCOLS + XCOLS), dtype=NP_F8)
        xq[:, :MCOLS] = mask
        xq[:, MCOLS:] = x5.transpose(1, 4, 2, 0, 3).reshape(128, XCOLS)
        in_maps.append({"xq": xq, "wse": wse})
    return in_maps


def kernel(x, weight, conv_bias, extra_bias):
    x = np.ascontiguousarray(np.asarray(x, dtype=np.float32))
    weight = np.ascontiguousarray(np.asarray(weight, dtype=np.float32))
    conv_bias = np.asarray(conv_bias, dtype=np.float32)
    extra_bias = np.asarray(extra_bias, dtype=np.float32)
    assert x.shape == (N, CIN, H, W), x.shape
    assert weight.shape == (CIN, COUT, K, K), weight.shape

    if "nc" not in _CACHE:
        _CACHE["nc"] = _build_module()
    nc = _CACHE["nc"]

    in_maps = _prep_inputs(x, weight, conv_bias, extra_bias)

    trace = os.environ.get("BASS_KERNEL_TRACE") == "1"
    res = run_bass_kernel_spmd(
        nc, in_maps, core_ids=list(range(NCORES)), trace=trace
    )
    _CACHE["last_result"] = res
    return np.concatenate([r["y"] for r in res.results], axis=0)


# revision 19
# speedup vs baseline: 1.0812x; 1.0021x over previous
# Trainium2 Bass kernel for: ConvTranspose2d(64->128, k=4, stride=1) -> spatial
# mean -> +biases -> 10*logsumexp over channels.
#
# Math: with full (K-1) output padding, the mean over the ENTIRE conv-transpose
# output spatial extent sees every input pixel through all K*K taps, so
#   pooled[n,co] = (sum_hw x[n,ci,hw]) @ (sum_kk w[ci,co,kk]) / (Ho*Wo) + cb + eb
# exactly. The conv collapses to a spatial sum + a (Cin x Cout) matmul.
#
# Sharding: data-parallel over batch N=32 across 8 cores (4 batches/core).
#
# Trace-driven design (see transcript):
# - x quantized to fp8 e4m3 on host (1 MiB/core, 4x less HBM traffic; final
#   output error ~1e-4 rel vs the 2e-2 gate since fp8 noise averages over the
#   4096-wide spatial sum).
# - Host pre-transposes x so (ci, hw%2) sits on partitions; the spatial sum
#   runs on the PE as a block-mask matmul. DoubleRow perf mode contracts two
#   256-column k-tiles per instruction (2 cols/cycle), so the PE tracks the
#   DMA stream even in the half-rate window the trace shows while SDMA writes
#   are in flight.
# - x rides BOTH HWDGE rings (2 chunks on SP, 2 on ACT) to reach the ~345 GB/s
#   HBM ceiling sooner; params go first on the ACT ring (they gated the first
#   matmul by 1.2us when queued behind the 1.3us ACT_TABLE_LOAD).
# - bias row is folded into the stage-2 matmul as a 65th contraction row of
#   wsum, removing a separate fp32 matmul (fp32 lowers to a slow LOW/HIGH
#   double pass on the PE).
# - One pre-placed LoadActFuncSet covering BOTH Exp and Ln (emitted after the
#   DMA issue instructions) keeps the 1.3us table load off the critical path.

import os

import ml_dtypes
import numpy as np

import concourse.bacc as bacc
import concourse.bass as bass
import concourse.mybir as mybir
import concourse.tile as tile
from concourse.bass_utils import run_bass_kernel_spmd
from concourse.hw_specs import get_activation_tables

N, CIN, COUT, K, H, W = 32, 64, 128, 4, 64, 64
NCORES = 8
NLOC = N // NCORES          # 4 batches per core
HW = H * W                  # 4096
SCALE = 1.0 / float((H + K - 1) * (W + K - 1))   # 1/4489

# x layout per core: xq[p, j], p = ci*2 + hw_lo, j = co_*256 + n*64 + ci_
# with hw = (co_*64 + ci_)*2 + hw_lo;  co_ = c_outer in [0,32), ci_ = c_inner.
COUT_CHUNKS = 32            # k-tiles accumulated in PSUM (c_outer)
CINNER = 64                 # folded by the DVE tail reduce
FD = NLOC * CINNER          # 256 columns per k-tile
XCOLS = COUT_CHUNKS * FD    # 8192
MCOLS = 2 * CIN             # mask columns embedded at the head of chunk 0
# x chunk sizes in DoubleRow-matmul units (512 cols = 64 KiB each) and the
# HWDGE ring each rides (s = SP/sync, a = ACT/scalar). Small chunks up front
# so the PE can start consuming during the slow (~150 GB/s) DMA ramp; two
# rings so descriptor gen (~0.65us each, serialized per engine) overlaps;
# sizes grow so cross-ring round-robin completions still land in PE program
# order; a tiny last chunk so the PE finishes right behind the stream.
MMS = [(1, "s"), (1, "a"), (2, "s"), (3, "a"), (4, "s"), (4, "a"), (1, "s")]

F32 = mybir.dt.float32
BF16 = mybir.dt.bfloat16
F8 = mybir.dt.float8e4
NP_F8 = ml_dtypes.float8_e4m3
NP_BF16 = ml_dtypes.bfloat16

_CACHE: dict = {}


def _build_module() -> bacc.Bacc:
    nc = bacc.Bacc("TRN2", target_bir_lowering=False, enable_partition_id=False)

    x_d = nc.dram_tensor("xq", [128, MCOLS + XCOLS], F8, kind="ExternalInput").ap()
    # wse padded to 128 partitions: a 65-partition DMA costs ~1.4us of
    # descriptor gen vs ~0.7us for a full-partition one.
    w_d = nc.dram_tensor("wse", [128, COUT], BF16, kind="ExternalInput").ap()
    y_d = nc.dram_tensor("y", [NLOC, 1], F32, kind="ExternalOutput").ap()

    with tile.TileContext(nc) as tc:
        with (
            tc.tile_pool(name="xpool", bufs=len(MMS)) as xpool,
            tc.tile_pool(name="small", bufs=1) as small,
            tc.tile_pool(name="ps1", bufs=1, space="PSUM") as ps1,
            tc.tile_pool(name="ps2", bufs=1, space="PSUM") as ps2,
        ):
            # One ACT table set covering BOTH Exp and Ln, pre-placed so the
            # insert_act_table_loads pass doesn't split them into two sets
            # and drop a 1.3us load between EXP and LN on the critical tail
            # (trace-verified in a run without this). The load is
            # non-blocking at program start.
            act_tables = get_activation_tables(nc.m.arch)
            set_id = next(
                i
                for i, (_, funcs) in enumerate(act_tables.items())
                if mybir.ActivationFunctionType.Exp in funcs
                and mybir.ActivationFunctionType.Ln in funcs
            )
            nc.scalar.add_instruction(
                mybir.InstLoadActFuncSet(
                    name=nc.get_next_instruction_name(), act_func_set_id=set_id
                )
            )

            # stage-2 lhsT: rows 0..63 get the spatial sums, row 64 is the
            # all-ones row that pulls in the bias row of wse.
            sT = small.tile([CIN + 1, NLOC], BF16)
            nc.vector.memset(sT[CIN : CIN + 1, :], 1.0)

            # ---- stage 1: spatial sums on the PE (fp8 DoubleRow) ----
            # The mask rides at the head of chunk 0 (a separate tiny-
            # descriptor param DMA completed 2us late and gated matmul 0).
            # P[ci, n*64 + ci_] accumulates sum over (hw_lo, c_outer).
            P = ps1.tile([CIN, FD], F32, space="PSUM")
            mask3 = None
            off = 0
            done = 0
            for k, (nmm, ring) in enumerate(MMS):
                cols = (MCOLS if k == 0 else 0) + nmm * 2 * FD
                xt = xpool.tile([128, cols], F8)
                eng = nc.sync if ring == "s" else nc.scalar
                eng.dma_start(out=xt, in_=x_d[:, off : off + cols])
                off += cols
                xoff = MCOLS if k == 0 else 0
                if k == 0:
                    mask3 = xt[:, 0:MCOLS].rearrange("p (k i) -> p k i", k=2)
                for c in range(nmm):
                    rhs3 = xt[
                        :, xoff + 2 * c * FD : xoff + 2 * (c + 1) * FD
                    ].rearrange("p (kk j) -> p kk j", kk=2)
                    nc.tensor.matmul(
                        out=P,
                        lhsT=mask3,
                        rhs=rhs3,
                        start=(done == 0),
                        stop=(done == COUT_CHUNKS // 2 - 1),
                        perf_mode=mybir.MatmulPerfMode.DoubleRow,
                    )
                    done += 1

            # wse behind the x chunks on the ACT ring; its data lands well
            # before the stage-2 matmul needs it.
            wset = small.tile([128, COUT], BF16)
            nc.scalar.dma_start(out=wset, in_=w_d)

            # ---- fold c_inner: sT[ci, n] = sum_ci_ P[ci, n*64+ci_] ----
            with nc.allow_low_precision(
                reason="S feeds a 64-deep bf16 matmul; fp8 input noise dominates"
            ):
                nc.vector.reduce_sum(
                    out=sT[0:CIN, :],
                    in_=P.rearrange("p (n c) -> p n c", n=NLOC),
                    axis=mybir.AxisListType.X,
                )

            # ---- stage 2: pooled[n, co] = sT.T @ wse (bias folded) ----
            pooled = ps2.tile([NLOC, COUT], F32, space="PSUM")
            nc.tensor.matmul(
                out=pooled, lhsT=sT, rhs=wset[0 : CIN + 1, :], start=True, stop=True
            )

            # ---- 10 * log(sum_co exp(pooled)) on ACT ----
            expt = small.tile([NLOC, COUT], F32)
            sume = small.tile([NLOC, 1], F32)
            nc.scalar.activation(
                out=expt,
                in_=pooled,
                func=mybir.ActivationFunctionType.Exp,
                accum_out=sume,
            )
            logv = small.tile([NLOC, 1], F32)
            nc.scalar.activation(
                out=logv, in_=sume, func=mybir.ActivationFunctionType.Ln
            )
            outv = small.tile([NLOC, 1], F32)
            nc.scalar.mul(out=outv, in_=logv, mul=10.0)
            # y rides the ACT ring: same engine as the preceding ops, so no
            # cross-engine semaphore hop before the final descriptor gen.
            nc.scalar.dma_start(out=y_d, in_=outv)

    nc.compile()
    return nc


def _prep_inputs(x, weight, conv_bias, extra_bias):
    wse = np.zeros((128, COUT), dtype=np.float32)
    wse[:CIN] = weight.sum(axis=(2, 3)) * SCALE
    wse[CIN] = conv_bias + extra_bias
    wse = wse.astype(NP_BF16)
    # mask[p, k*64 + i] = (p//2 == i), duplicated over the two k-tiles
    mask = np.zeros((128, MCOLS), dtype=NP_F8)
    for kk in range(2):
        mask[np.arange(128), kk * CIN + np.arange(128) // 2] = 1.0
    in_maps = []
    for c in range(NCORES):
        xs = x[c * NLOC : (c + 1) * NLOC]                          # (4,64,64,64)
        # (n, ci, co_, ci_, hw_lo) -> (ci, hw_lo, co_, n, ci_)
        x5 = xs.reshape(NLOC, CIN, COUT_CHUNKS, CINNER, 2)
        xq = np.empty((128, MCOLS + XCOLS), dtype=NP_F8)
        xq[:, :MCOLS] = mask
        xq[:, MCOLS:] = x5.transpose(1, 4, 2, 0, 3).reshape(128, XCOLS)
        in_maps.append({"xq": xq, "wse": wse})
    return in_maps


def kernel(x, weight, conv_bias, extra_bias):
    x = np.ascontiguousarray(np.asarray(x, dtype=np.float32))
    weight = np.ascontiguousarray(np.asarray(weight, dtype=np.float32))
    conv_bias = np.asarray(conv_bias, dtype=np.float32)
    extra_bias = np.asarray(extra_bias, dtype=np.float32)
    assert x.shape == (N, CIN, H, W), x.shape
    assert weight.shape == (CIN, COUT, K, K), weight.shape

    if "nc" not in _CACHE:
        _CACHE["nc"] = _build_module()
    nc = _CACHE["nc"]

    in_maps = _prep_inputs(x, weight, conv_bias, extra_bias)

    trace = os.environ.get("BASS_KERNEL_TRACE") == "1"
    res = run_bass_kernel_spmd(
        nc, in_maps, core_ids=list(range(NCORES)), trace=trace
    )
    _CACHE["last_result"] = res
    return np.concatenate([r["y"] for r in res.results], axis=0)


# revision 20
# speedup vs baseline: 1.0914x; 1.0094x over previous
# Trainium2 Bass kernel for: ConvTranspose2d(64->128, k=4, stride=1) -> spatial
# mean -> +biases -> 10*logsumexp over channels.
#
# Math: with full (K-1) output padding, the mean over the ENTIRE conv-transpose
# output spatial extent sees every input pixel through all K*K taps, so
#   pooled[n,co] = (sum_hw x[n,ci,hw]) @ (sum_kk w[ci,co,kk]) / (Ho*Wo) + cb + eb
# exactly. The conv collapses to a spatial sum + a (Cin x Cout) matmul.
#
# Sharding: data-parallel over batch N=32 across 8 cores (4 batches/core).
#
# Trace-driven design (see transcript):
# - x quantized to fp8 e4m3 on host (1 MiB/core, 4x less HBM traffic; final
#   output error ~1e-4 rel vs the 2e-2 gate since fp8 noise averages over the
#   4096-wide spatial sum).
# - Host pre-transposes x so (ci, hw%2) sits on partitions; the spatial sum
#   runs on the PE as a block-mask matmul. DoubleRow perf mode contracts two
#   256-column k-tiles per instruction (2 cols/cycle), so the PE tracks the
#   DMA stream even in the half-rate window the trace shows while SDMA writes
#   are in flight.
# - x rides BOTH HWDGE rings (2 chunks on SP, 2 on ACT) to reach the ~345 GB/s
#   HBM ceiling sooner; params go first on the ACT ring (they gated the first
#   matmul by 1.2us when queued behind the 1.3us ACT_TABLE_LOAD).
# - bias row is folded into the stage-2 matmul as a 65th contraction row of
#   wsum, removing a separate fp32 matmul (fp32 lowers to a slow LOW/HIGH
#   double pass on the PE).
# - One pre-placed LoadActFuncSet covering BOTH Exp and Ln (emitted after the
#   DMA issue instructions) keeps the 1.3us table load off the critical path.

import os

import ml_dtypes
import numpy as np

import concourse.bacc as bacc
import concourse.bass as bass
import concourse.mybir as mybir
import concourse.tile as tile
from concourse.bass_utils import run_bass_kernel_spmd
from concourse.hw_specs import get_activation_tables

N, CIN, COUT, K, H, W = 32, 64, 128, 4, 64, 64
NCORES = 8
NLOC = N // NCORES          # 4 batches per core
HW = H * W                  # 4096
SCALE = 1.0 / float((H + K - 1) * (W + K - 1))   # 1/4489

# x layout per core: xq[p, j], p = ci*2 + hw_lo, j = co_*256 + n*64 + ci_
# with hw = (co_*64 + ci_)*2 + hw_lo;  co_ = c_outer in [0,32), ci_ = c_inner.
COUT_CHUNKS = 32            # k-tiles accumulated in PSUM (c_outer)
CINNER = 64                 # folded by the DVE tail reduce
FD = NLOC * CINNER          # 256 columns per k-tile
XCOLS = COUT_CHUNKS * FD    # 8192
MCOLS = 2 * CIN             # mask columns embedded at the head of chunk 0
# x chunk sizes in DoubleRow-matmul units (512 cols = 64 KiB each) and the
# HWDGE ring each rides (s = SP/sync, a = ACT/scalar). Small chunks up front
# so the PE can start consuming during the slow (~150 GB/s) DMA ramp; two
# rings so descriptor gen (~0.65us each, serialized per engine) overlaps;
# sizes grow so cross-ring round-robin completions still land in PE program
# order; a tiny last chunk so the PE finishes right behind the stream.
MMS = [(1, "s"), (3, "s"), (5, "s"), (5, "s"), (2, "s")]

F32 = mybir.dt.float32
BF16 = mybir.dt.bfloat16
F8 = mybir.dt.float8e4
NP_F8 = ml_dtypes.float8_e4m3
NP_BF16 = ml_dtypes.bfloat16

_CACHE: dict = {}


def _build_module() -> bacc.Bacc:
    nc = bacc.Bacc("TRN2", target_bir_lowering=False, enable_partition_id=False)

    x_d = nc.dram_tensor("xq", [128, MCOLS + XCOLS], F8, kind="ExternalInput").ap()
    # wse padded to 128 partitions: a 65-partition DMA costs ~1.4us of
    # descriptor gen vs ~0.7us for a full-partition one.
    w_d = nc.dram_tensor("wse", [128, COUT], BF16, kind="ExternalInput").ap()
    y_d = nc.dram_tensor("y", [NLOC, 1], F32, kind="ExternalOutput").ap()

    with tile.TileContext(nc) as tc:
        with (
            tc.tile_pool(name="xpool", bufs=len(MMS)) as xpool,
            tc.tile_pool(name="small", bufs=1) as small,
            tc.tile_pool(name="ps1", bufs=1, space="PSUM") as ps1,
            tc.tile_pool(name="ps2", bufs=1, space="PSUM") as ps2,
        ):
            # One ACT table set covering BOTH Exp and Ln, pre-placed so the
            # insert_act_table_loads pass doesn't split them into two sets
            # and drop a 1.3us load between EXP and LN on the critical tail
            # (trace-verified in a run without this). The load is
            # non-blocking at program start.
            act_tables = get_activation_tables(nc.m.arch)
            set_id = next(
                i
                for i, (_, funcs) in enumerate(act_tables.items())
                if mybir.ActivationFunctionType.Exp in funcs
                and mybir.ActivationFunctionType.Ln in funcs
            )
            nc.scalar.add_instruction(
                mybir.InstLoadActFuncSet(
                    name=nc.get_next_instruction_name(), act_func_set_id=set_id
                )
            )

            # stage-2 lhsT: rows 0..63 get the spatial sums, row 64 is the
            # all-ones row that pulls in the bias row of wse.
            sT = small.tile([CIN + 1, NLOC], BF16)
            nc.vector.memset(sT[CIN : CIN + 1, :], 1.0)

            # ---- stage 1: spatial sums on the PE (fp8 DoubleRow) ----
            # The mask rides at the head of chunk 0 (a separate tiny-
            # descriptor param DMA completed 2us late and gated matmul 0).
            # P[ci, n*64 + ci_] accumulates sum over (hw_lo, c_outer).
            P = ps1.tile([CIN, FD], F32, space="PSUM")
            mask3 = None
            off = 0
            done = 0
            for k, (nmm, ring) in enumerate(MMS):
                cols = (MCOLS if k == 0 else 0) + nmm * 2 * FD
                xt = xpool.tile([128, cols], F8)
                eng = nc.sync if ring == "s" else nc.scalar
                eng.dma_start(out=xt, in_=x_d[:, off : off + cols])
                off += cols
                xoff = MCOLS if k == 0 else 0
                if k == 0:
                    mask3 = xt[:, 0:MCOLS].rearrange("p (k i) -> p k i", k=2)
                for c in range(nmm):
                    rhs3 = xt[
                        :, xoff + 2 * c * FD : xoff + 2 * (c + 1) * FD
                    ].rearrange("p (kk j) -> p kk j", kk=2)
                    nc.tensor.matmul(
                        out=P,
                        lhsT=mask3,
                        rhs=rhs3,
                        start=(done == 0),
                        stop=(done == COUT_CHUNKS // 2 - 1),
                        perf_mode=mybir.MatmulPerfMode.DoubleRow,
                    )
                    done += 1

            # wse behind the x chunks on the ACT ring; its data lands well
            # before the stage-2 matmul needs it.
            wset = small.tile([128, COUT], BF16)
            nc.scalar.dma_start(out=wset, in_=w_d)

            # ---- fold c_inner: sT[ci, n] = sum_ci_ P[ci, n*64+ci_] ----
            with nc.allow_low_precision(
                reason="S feeds a 64-deep bf16 matmul; fp8 input noise dominates"
            ):
                nc.vector.reduce_sum(
                    out=sT[0:CIN, :],
                    in_=P.rearrange("p (n c) -> p n c", n=NLOC),
                    axis=mybir.AxisListType.X,
                )

            # ---- stage 2: pooled[n, co] = sT.T @ wse (bias folded) ----
            pooled = ps2.tile([NLOC, COUT], F32, space="PSUM")
            nc.tensor.matmul(
                out=pooled, lhsT=sT, rhs=wset[0 : CIN + 1, :], start=True, stop=True
            )

            # ---- 10 * log(sum_co exp(pooled)) on ACT ----
            expt = small.tile([NLOC, COUT], F32)
            sume = small.tile([NLOC, 1], F32)
            nc.scalar.activation(
                out=expt,
                in_=pooled,
                func=mybir.ActivationFunctionType.Exp,
                accum_out=sume,
            )
            logv = small.tile([NLOC, 1], F32)
            nc.scalar.activation(
                out=logv, in_=sume, func=mybir.ActivationFunctionType.Ln
            )
            outv = small.tile([NLOC, 1], F32)
            nc.scalar.mul(out=outv, in_=logv, mul=10.0)
            # y rides the ACT ring: same engine as the preceding ops, so no
            # cross-engine semaphore hop before the final descriptor gen.
            nc.scalar.dma_start(out=y_d, in_=outv)

    nc.compile()
    return nc


def _prep_inputs(x, weight, conv_bias, extra_bias):
    wse = np.zeros((128, COUT), dtype=np.float32)
    wse[:CIN] = weight.sum(axis=(2, 3)) * SCALE
    wse[CIN] = conv_bias + extra_bias
    wse = wse.astype(NP_BF16)
    # mask[p, k*64 + i] = (p//2 == i), duplicated over the two k-tiles
    mask = np.zeros((128, MCOLS), dtype=NP_F8)
    for kk in range(2):
        mask[np.arange(128), kk * CIN + np.arange(128) // 2] = 1.0
    in_maps = []
    for c in range(NCORES):
        xs = x[c * NLOC : (c + 1) * NLOC]                          # (4,64,64,64)
        # (n, ci, co_, ci_, hw_lo) -> (ci, hw_lo, co_, n, ci_)
        x5 = xs.reshape(NLOC, CIN, COUT_CHUNKS, CINNER, 2)
        xq = np.empty((128, MCOLS + XCOLS), dtype=NP_F8)
        xq[:, :MCOLS] = mask
        xq[:, MCOLS:] = x5.transpose(1, 4, 2, 0, 3).reshape(128, XCOLS)
        in_maps.append({"xq": xq, "wse": wse})
    return in_maps


def kernel(x, weight, conv_bias, extra_bias):
    x = np.ascontiguousarray(np.asarray(x, dtype=np.float32))
    weight = np.ascontiguousarray(np.asarray(weight, dtype=np.float32))
    conv_bias = np.asarray(conv_bias, dtype=np.float32)
    extra_bias = np.asarray(extra_bias, dtype=np.float32)
    assert x.shape == (N, CIN, H, W), x.shape
    assert weight.shape == (CIN, COUT, K, K), weight.shape

    if "nc" not in _CACHE:
        _CACHE["nc"] = _build_module()
    nc = _CACHE["nc"]

    in_maps = _prep_inputs(x, weight, conv_bias, extra_bias)

    trace = os.environ.get("BASS_KERNEL_TRACE") == "1"
    res = run_bass_kernel_spmd(
        nc, in_maps, core_ids=list(range(NCORES)), trace=trace
    )
    _CACHE["last_result"] = res
    return np.concatenate([r["y"] for r in res.results], axis=0)
